# revision 3
# baseline (speedup 1.0000x reference)
"""MultiHeadAttention forward on 8 TRN2 NeuronCores (batch*head parallel).

Problem: S=2048, B=2, E=1024, H=16 heads, D=64. Each core handles one batch
(b = core//4) and 4 consecutive heads ((core%4)*4 ...). Per-core program:

  Phase A: QKV projection. Host pre-transposes x and the weight slices so the
    contraction dim (E) lands on SBUF partitions. Q^T/K^T computed in
    feature-major layout [f, s]; V computed in natural [s, d] layout with an
    appended ones column (softmax denominator trick).
  Phase B: per head-pair (row-packed K=64 matmuls), per t-half (1024):
    scores^T tiles [s-chunk 128, 1024] -> ACT exp (scale=1/8 folded) ->
    P@V accumulation with [V|1] stationary -> denominators at partition 64.
    Normalization: DVE reciprocal + PE ones-broadcast + DVE multiply.
  Phase C: out_proj partial (core's 256 embed dims x full 1024 out features);
    host sums 4 partials per batch and adds out_proj_bias.

All matmuls run in float32r (TF32-like, 1 cycle/row at N>=256, ~1e-3 rel err).
"""
import sys

if "/opt/trn_rl_repo" not in sys.path:
    sys.path.insert(0, "/opt/trn_rl_repo")

import numpy as np

import concourse.bass as bass
import concourse.tile as tile
from concourse import mybir
from concourse.bass_utils import run_bass_kernel_spmd

S = 2048
B = 2
E = 1024
H = 16
D = 64
HEADS_PER_CORE = 4
N_CORES = 8
F32 = mybir.dt.float32
F32R = mybir.dt.float32r
EXP = mybir.ActivationFunctionType.Exp
SCALING = float(D) ** -0.5

NSCH = S // 128   # 16 s-chunks
NSB = S // 512    # 4 s-blocks
NEC = E // 128    # 8 e-chunks


def _split_excess_waits(nc, limit=1):
    """This walrus build accepts at most 2 sync-wait commands per instruction;
    hoist excess waits onto preceding same-engine NOPs (queue order preserves
    semantics)."""
    ctr = 0
    for f in nc.m.functions:
        for blk in f.blocks:
            insts = blk.instructions
            if not any(
                i.sync_info and i.sync_info.on_wait and len(i.sync_info.on_wait) > limit
                for i in insts
            ):
                continue
            out = []
            for inst in insts:
                si = inst.sync_info
                if si and si.on_wait and len(si.on_wait) > limit:
                    waits = list(si.on_wait)
                    excess, keep = waits[:-limit], waits[-limit:]
                    for i in range(0, len(excess), limit):
                        ctr += 1
                        nop = mybir.InstNoOp(name=f"waitsplit-nop-{ctr}")
                        nop.engine = inst.engine
                        nop.sync_info = mybir.SyncInfo(
                            on_wait=excess[i : i + limit], on_update=[]
                        )
                        nc.register_instruction(nop, overwrite=True)
                        out.append(nop)
                    si.on_wait = keep
                out.append(inst)
            blk.instructions.clear()
            blk.instructions.extend(out)
    return nc


def _build_nc():
    nc = bass.Bass()
    xT = nc.dram_tensor("xT", [E, S], F32, kind="ExternalInput")
    wqkT = nc.dram_tensor("wqkT", [E, 512], F32, kind="ExternalInput")
    wvT = nc.dram_tensor("wvT", [E, 256], F32, kind="ExternalInput")
    woutT = nc.dram_tensor("woutT", [256, E], F32, kind="ExternalInput")
    bias_qk = nc.dram_tensor("bias_qk", [128, 4], F32, kind="ExternalInput")
    bias_v = nc.dram_tensor("bias_v", [1, 256], F32, kind="ExternalInput")
    outT = nc.dram_tensor("outT", [E, S], F32, kind="ExternalOutput")

    with tile.TileContext(nc) as tc:
        with tc.tile_pool(name="wpool", bufs=1) as wpool, \
             tc.tile_pool(name="qkpool", bufs=1) as qkpool, \
             tc.tile_pool(name="vapool", bufs=1) as vapool, \
             tc.tile_pool(name="attnpool", bufs=1) as attnpool:
            # ---- constants / weights ----
            wqk = wpool.tile([128, NEC, 512], F32R)
            nc.gpsimd.dma_start(
                out=wqk, in_=wqkT.rearrange("(c p) f -> p c f", p=128))
            wv = wpool.tile([128, NEC, 256], F32R)
            nc.gpsimd.dma_start(
                out=wv, in_=wvT.rearrange("(c p) f -> p c f", p=128))
            wout = wpool.tile([128, 2, E], F32R)
            nc.gpsimd.dma_start(
                out=wout, in_=woutT.rearrange("(c p) f -> p c f", p=128))
            bqk = wpool.tile([128, 4], F32)
            nc.sync.dma_start(out=bqk, in_=bias_qk[:, :])
            bv = wpool.tile([128, 256], F32)
            nc.sync.dma_start(out=bv, in_=bias_v[:, :].to_broadcast([128, 256]))
            ones64_32 = wpool.tile([128, 64], F32)
            nc.vector.memset(ones64_32, 1.0)
            onesbc32 = wpool.tile([1, 64], F32)
            nc.vector.memset(onesbc32, 1.0)
            onesbc = wpool.tile([1, 64], F32R)
            nc.vector.tensor_copy(onesbc, onesbc32)

            # persistent activations
            qk = qkpool.tile([128, 4, S], F32R)       # Q^T (chunks 0-1), K^T (2-3)
            va = vapool.tile([128, NSCH, 4, 65], F32R)  # V natural + ones col
            attn = attnpool.tile([128, 2, S], F32R)   # attn^T, heads packed

            # ones columns of va (fp32 -> f32r cast on write)
            nc.vector.tensor_copy(
                va[:, :, :, 64:65],
                ones64_32.rearrange("p (c h) -> p c h", h=4).unsqueeze(3))

            # ---- phase A: QKV projection ----
            with tc.tile_pool(name="xpool", bufs=1) as xpool, \
                 tc.tile_pool(name="apsum", bufs=2, space="PSUM") as apsum:
                xt = xpool.tile([128, NEC, S], F32R)
                nc.gpsimd.dma_start(
                    out=xt, in_=xT.rearrange("(c p) s -> p c s", p=128))

                # Q^T / K^T: [f-chunk 128, s-block 512]
                for fc in range(4):
                    for sb in range(NSB):
                        ps = apsum.tile([128, 512], F32, tag="qkps")
                        for ec in range(NEC):
                            nc.tensor.matmul(
                                ps,
                                wqk[:, ec, bass.ts(fc, 128)],
                                xt[:, ec, bass.ts(sb, 512)],
                                start=(ec == 0), stop=(ec == NEC - 1))
                        nc.vector.tensor_scalar(
                            out=qk[:, fc, bass.ts(sb, 512)], in0=ps,
                            scalar1=bqk[:, fc:fc + 1], scalar2=None,
                            op0=mybir.AluOpType.add)

                # V natural: per s-chunk [128 s, 256 d]
                for i in range(NSCH):
                    ps = apsum.tile([128, 256], F32, tag="vps")
                    for ec in range(NEC):
                        nc.tensor.matmul(
                            ps,
                            xt[:, ec, bass.ts(i, 128)],
                            wv[:, ec, :],
                            start=(ec == 0), stop=(ec == NEC - 1))
                    nc.vector.tensor_tensor(
                        out=va[:, i, :, 0:64],
                        in0=ps.rearrange("p (h d) -> p h d", h=4),
                        in1=bv.rearrange("p (h d) -> p h d", h=4),
                        op=mybir.AluOpType.add)

            # ---- phase B: attention per head pair, per t-half ----
            with tc.tile_pool(name="ppool", bufs=4) as ppool, \
                 tc.tile_pool(name="fpool", bufs=2) as fpool, \
                 tc.tile_pool(name="scpsum", bufs=1, space="PSUM") as scp, \
                 tc.tile_pool(name="pvpsum", bufs=1, space="PSUM") as pvp:
                for pair in range(2):
                    hA, hB = 2 * pair, 2 * pair + 1
                    qc = pair       # Q chunk for this pair
                    kc = 2 + pair   # K chunk
                    for th in range(2):
                        toff = th * 1024
                        pvA = pvp.tile([65, 1024], F32, tag="pvA")
                        pvB = pvp.tile([65, 1024], F32, tag="pvB")
                        for i in range(NSCH):
                            scA = scp.tile([128, 1024], F32, tag="scA")
                            scB = scp.tile([128, 1024], F32, tag="scB")
                            for j in range(2):
                                nc.tensor.matmul(
                                    scA[:, bass.ts(j, 512)],
                                    qk[0:64, kc, bass.ts(i, 128)],
                                    qk[0:64, qc, bass.ds(toff + j * 512, 512)],
                                    start=True, stop=True)
                                nc.tensor.matmul(
                                    scB[:, bass.ts(j, 512)],
                                    qk[64:128, kc, bass.ts(i, 128)],
                                    qk[64:128, qc, bass.ds(toff + j * 512, 512)],
                                    start=True, stop=True)
                            pA = ppool.tile([128, 1024], F32R, tag="p")
                            pB = ppool.tile([128, 1024], F32R, tag="p")
                            nc.scalar.activation(pA, scA, EXP, scale=SCALING)
                            nc.scalar.activation(pB, scB, EXP, scale=SCALING)
                            for j in range(2):
                                nc.tensor.matmul(
                                    pvA[:, bass.ts(j, 512)],
                                    va[:, i, hA, :],
                                    pA[:, bass.ts(j, 512)],
                                    start=(i == 0), stop=(i == NSCH - 1))
                                nc.tensor.matmul(
                                    pvB[:, bass.ts(j, 512)],
                                    va[:, i, hB, :],
                                    pB[:, bass.ts(j, 512)],
                                    start=(i == 0), stop=(i == NSCH - 1))
                        # finalize: normalize by the ones-column sums
                        for h, pv in ((hA, pvA), (hB, pvB)):
                            prt = (h % 2) * 64
                            rec = fpool.tile([1, 1024], F32, tag="rec")
                            nc.vector.reciprocal(rec, pv[64:65, :])
                            recr = fpool.tile([1, 1024], F32R, tag="recr")
                            nc.vector.tensor_copy(recr, rec)
                            for j in range(2):
                                bc = scp.tile([64, 512], F32, tag="scA")
                                nc.tensor.matmul(
                                    bc, onesbc, recr[:, bass.ts(j, 512)],
                                    start=True, stop=True)
                                bcs = fpool.tile([64, 512], F32, tag="bcs")
                                nc.vector.tensor_copy(bcs, bc)
                                nc.vector.tensor_mul(
                                    attn[prt:prt + 64, pair,
                                         bass.ds(toff + j * 512, 512)],
                                    pv[0:64, bass.ts(j, 512)],
                                    bcs)

            # ---- phase C: out_proj partial ----
            with tc.tile_pool(name="opool", bufs=2) as opool, \
                 tc.tile_pool(name="cpsum", bufs=4, space="PSUM") as cpsum:
                for fc in range(NEC):
                    ost = opool.tile([128, S], F32, tag="ost")
                    for sb in range(NSB):
                        ps = cpsum.tile([128, 512], F32, tag="ops")
                        for ec in range(2):
                            nc.tensor.matmul(
                                ps,
                                wout[:, ec, bass.ts(fc, 128)],
                                attn[:, ec, bass.ts(sb, 512)],
                                start=(ec == 0), stop=(ec == 1))
                        nc.vector.tensor_copy(ost[:, bass.ts(sb, 512)], ps)
                    nc.sync.dma_start(
                        out=outT[bass.ts(fc, 128), :], in_=ost)
    _split_excess_waits(nc)
    return nc


_NC_CACHE = None


def _get_nc():
    global _NC_CACHE
    if _NC_CACHE is None:
        _NC_CACHE = _build_nc()
    return _NC_CACHE


def kernel(x, in_proj_weight, in_proj_bias, out_proj_weight, out_proj_bias,
           _run_kwargs=None, _capture=None):
    x = np.asarray(x, dtype=np.float32)
    in_proj_weight = np.asarray(in_proj_weight, dtype=np.float32)
    in_proj_bias = np.asarray(in_proj_bias, dtype=np.float32)
    out_proj_weight = np.asarray(out_proj_weight, dtype=np.float32)
    out_proj_bias = np.asarray(out_proj_bias, dtype=np.float32)

    nc = _get_nc()
    xTb = [np.ascontiguousarray(x[:, b, :].T) for b in range(B)]

    in_maps = []
    for c in range(N_CORES):
        b = c // 4
        h0 = (c % 4) * HEADS_PER_CORE
        rows = slice(h0 * D, h0 * D + HEADS_PER_CORE * D)
        wq = in_proj_weight[0:E][rows]          # [256, 1024]
        wk = in_proj_weight[E:2 * E][rows]
        wv_ = in_proj_weight[2 * E:3 * E][rows]
        wqkT = np.ascontiguousarray(np.concatenate([wq, wk], axis=0).T)  # [1024, 512]
        wvT = np.ascontiguousarray(wv_.T)       # [1024, 256]
        woutT = np.ascontiguousarray(out_proj_weight[:, rows].T)  # [256, 1024]
        bqk = np.concatenate(
            [in_proj_bias[0:E][rows], in_proj_bias[E:2 * E][rows]])  # [512]
        bias_qk = np.ascontiguousarray(bqk.reshape(4, 128).T)  # [128, 4]
        bias_v = in_proj_bias[2 * E:3 * E][rows].reshape(1, 256)
        in_maps.append({
            "xT": xTb[b],
            "wqkT": wqkT,
            "wvT": wvT,
            "woutT": woutT,
            "bias_qk": np.ascontiguousarray(bias_qk),
            "bias_v": np.ascontiguousarray(bias_v),
        })

    res = run_bass_kernel_spmd(nc, in_maps, core_ids=list(range(N_CORES)),
                               **(_run_kwargs or {}))
    if _capture is not None:
        _capture["res"] = res

    out = np.zeros((S, B, E), dtype=np.float32)
    for c in range(N_CORES):
        b = c // 4
        out[:, b, :] += res.results[c]["outT"].T
    out += out_proj_bias
    return out


# revision 5
# speedup vs baseline: 1.2886x; 1.2886x over previous
"""MultiHeadAttention forward on 8 TRN2 NeuronCores (batch*head parallel).

Problem: S=2048, B=2, E=1024, H=16 heads, D=64. Each core handles one batch
(b = core//4) and 4 consecutive heads ((core%4)*4 ...), as 2 head-pairs.

Per-core program (all matmuls float32r: 1 cycle/row at N>=256, ~1e-3 rel err):
  Phase A: QKV projection. Host pre-transposes x and weight slices so the
    contraction dim (E) lands on SBUF partitions. Q^T/K^T in feature-major
    [f, s] layout; V in natural [s, d] layout with an appended ones column
    (the softmax denominator drops out of the P@V matmul).
  Phase B: per head-pair, per t-quarter (512): row-packed K=64 score matmuls
    (heads at array rows 0-63/64-127 run concurrently), one ACT exp over the
    combined [128, 1024] PSUM strip (scale=1/8 folded in), P@V accumulation
    with [V|1] stationary, then a PSUM->SBUF staging copy (frees the PSUM
    bank immediately; normalization is deferred).
  Finalize (per pair, off critical path): ONE batched DVE reciprocal over all
    8 denominator rows, PE ones-broadcast, DVE multiply.
  Phase C: out_proj partials per pair (pair 0 overlaps phase B of pair 1);
    host sums the 2x4 partials per batch and adds out_proj_bias.
"""
import sys

if "/opt/trn_rl_repo" not in sys.path:
    sys.path.insert(0, "/opt/trn_rl_repo")

import numpy as np

import concourse.bass as bass
import concourse.tile as tile
from concourse import mybir
from concourse.bass_utils import run_bass_kernel_spmd

S = 2048
B = 2
E = 1024
H = 16
D = 64
N_CORES = 8
F32 = mybir.dt.float32
F32R = mybir.dt.float32r
EXP = mybir.ActivationFunctionType.Exp
SCALING = float(D) ** -0.5

NSCH = S // 128   # 16 s-chunks
NSB = S // 512    # 4 s-blocks
NEC = E // 128    # 8 e-chunks


def _split_excess_waits(nc, limit=1):
    """This walrus build accepts at most 2 sync-wait commands per instruction;
    hoist excess waits onto preceding same-engine NOPs (queue order preserves
    semantics)."""
    ctr = 0
    for f in nc.m.functions:
        for blk in f.blocks:
            insts = blk.instructions
            if not any(
                i.sync_info and i.sync_info.on_wait and len(i.sync_info.on_wait) > limit
                for i in insts
            ):
                continue
            out = []
            for inst in insts:
                si = inst.sync_info
                if si and si.on_wait and len(si.on_wait) > limit:
                    waits = list(si.on_wait)
                    excess, keep = waits[:-limit], waits[-limit:]
                    for i in range(0, len(excess), limit):
                        ctr += 1
                        nop = mybir.InstNoOp(name=f"waitsplit-nop-{ctr}")
                        nop.engine = inst.engine
                        nop.sync_info = mybir.SyncInfo(
                            on_wait=excess[i : i + limit], on_update=[]
                        )
                        nc.register_instruction(nop, overwrite=True)
                        out.append(nop)
                    si.on_wait = keep
                out.append(inst)
            blk.instructions.clear()
            blk.instructions.extend(out)
    return nc


def _build_nc():
    nc = bass.Bass()
    # f32r-typed DRAM inputs: numpy float32 bytes, HWDGE plain DMA, PE rounds.
    xT = nc.dram_tensor("xT", [E, S], F32R, kind="ExternalInput")
    wqkT = nc.dram_tensor("wqkT", [E, 512], F32R, kind="ExternalInput")
    wvT = nc.dram_tensor("wvT", [E, 256], F32R, kind="ExternalInput")
    woutT = nc.dram_tensor("woutT", [256, E], F32R, kind="ExternalInput")
    bias_qk = nc.dram_tensor("bias_qk", [128, 4], F32, kind="ExternalInput")
    bias_v = nc.dram_tensor("bias_v", [1, 256], F32, kind="ExternalInput")
    outT = nc.dram_tensor("outT", [2, E, S], F32, kind="ExternalOutput")

    with tile.TileContext(nc) as tc:
        with tc.tile_pool(name="wpool", bufs=1) as wpool, \
             tc.tile_pool(name="qkpool", bufs=1) as qkpool, \
             tc.tile_pool(name="vapool", bufs=1) as vapool, \
             tc.tile_pool(name="attnpool", bufs=1) as attnpool, \
             tc.tile_pool(name="auxpsum", bufs=2, space="PSUM") as auxp:
            # ---- constants / weights ----
            wqk = wpool.tile([128, NEC, 512], F32R)
            nc.sync.dma_start(
                out=wqk, in_=wqkT.rearrange("(c p) f -> p c f", p=128))
            wv = wpool.tile([128, NEC, 256], F32R)
            nc.sync.dma_start(
                out=wv, in_=wvT.rearrange("(c p) f -> p c f", p=128))
            wout = wpool.tile([128, 2, E], F32R)
            nc.sync.dma_start(
                out=wout, in_=woutT.rearrange("(c p) f -> p c f", p=128))
            bqk = wpool.tile([128, 4], F32)
            nc.sync.dma_start(out=bqk, in_=bias_qk[:, :])
            bv = wpool.tile([128, 256], F32)
            nc.sync.dma_start(out=bv, in_=bias_v[:, :].to_broadcast([128, 256]))
            ones64_32 = wpool.tile([128, 64], F32)
            nc.vector.memset(ones64_32, 1.0)

            # persistent activations
            qk = qkpool.tile([128, 4, S], F32R)        # Q^T (chunks 0-1), K^T (2-3)
            va = vapool.tile([128, NSCH, 4, 65], F32R)  # V natural + ones col
            attn = attnpool.tile([128, 2, S], F32R)    # attn^T normalized

            nc.vector.tensor_copy(
                va[:, :, :, 64:65],
                ones64_32.rearrange("p (c h) -> p c h", h=4).unsqueeze(3))

            # ---- phase A: QKV projection ----
            with nc.named_scope("phaseA"), \
                 tc.tile_pool(name="xpool", bufs=1) as xpool, \
                 tc.tile_pool(name="apsum", bufs=2, space="PSUM") as apsum:
                xt = xpool.tile([128, NEC, S], F32R)
                for ec in range(NEC):
                    nc.sync.dma_start(
                        out=xt[:, ec, :], in_=xT[bass.ts(ec, 128), :])

                for fc in range(4):
                    for sb in range(NSB):
                        ps = apsum.tile([128, 512], F32, tag="qkps")
                        for ec in range(NEC):
                            nc.tensor.matmul(
                                ps,
                                wqk[:, ec, bass.ts(fc, 128)],
                                xt[:, ec, bass.ts(sb, 512)],
                                start=(ec == 0), stop=(ec == NEC - 1))
                        nc.vector.tensor_scalar(
                            out=qk[:, fc, bass.ts(sb, 512)], in0=ps,
                            scalar1=bqk[:, fc:fc + 1], scalar2=None,
                            op0=mybir.AluOpType.add)

                for i in range(NSCH):
                    ps = apsum.tile([128, 256], F32, tag="vps")
                    for ec in range(NEC):
                        nc.tensor.matmul(
                            ps,
                            xt[:, ec, bass.ts(i, 128)],
                            wv[:, ec, :],
                            start=(ec == 0), stop=(ec == NEC - 1))
                    nc.vector.tensor_tensor(
                        out=va[:, i, :, 0:64],
                        in0=ps.rearrange("p (h d) -> p h d", h=4),
                        in1=bv.rearrange("p (h d) -> p h d", h=4),
                        op=mybir.AluOpType.add)

            # ---- phase B + finalize + out_proj, pair by pair ----
            with tc.tile_pool(name="ppool", bufs=3) as ppool, \
                 tc.tile_pool(name="unpool", bufs=2) as unpool, \
                 tc.tile_pool(name="fpool", bufs=1) as fpool, \
                 tc.tile_pool(name="opool", bufs=2) as opool, \
                 tc.tile_pool(name="scpsum", bufs=2, space="PSUM") as scp, \
                 tc.tile_pool(name="pvpsum", bufs=1, space="PSUM") as pvp:
                for pair in range(2):
                    hA, hB = 2 * pair, 2 * pair + 1
                    qc = pair       # Q chunk of this pair
                    kc = 2 + pair   # K chunk
                    # unnormalized P@V results + sums: 8 slots of [65, 512]
                    # (slot = tq*2 + head), row 64 = denominators
                    un = unpool.tile([65, 8, 512], F32, tag="un")
                    with nc.named_scope(f"scores{pair}"):
                        for tq in range(4):
                            toff = tq * 512
                            pvA = pvp.tile([65, 512], F32, tag="pvA")
                            pvB = pvp.tile([65, 512], F32, tag="pvB")
                            for i in range(NSCH):
                                sc = scp.tile([128, 1024], F32, tag="sc")
                                nc.tensor.matmul(
                                    sc[:, 0:512],
                                    qk[0:64, kc, bass.ts(i, 128)],
                                    qk[0:64, qc, bass.ds(toff, 512)],
                                    start=True, stop=True)
                                nc.tensor.matmul(
                                    sc[:, 512:1024],
                                    qk[64:128, kc, bass.ts(i, 128)],
                                    qk[64:128, qc, bass.ds(toff, 512)],
                                    start=True, stop=True)
                                p = ppool.tile([128, 1024], F32R, tag="p")
                                nc.scalar.activation(p, sc, EXP, scale=SCALING)
                                nc.tensor.matmul(
                                    pvA, va[:, i, hA, :], p[:, 0:512],
                                    start=(i == 0), stop=(i == NSCH - 1))
                                nc.tensor.matmul(
                                    pvB, va[:, i, hB, :], p[:, 512:1024],
                                    start=(i == 0), stop=(i == NSCH - 1))
                            nc.vector.tensor_copy(un[:, tq * 2, :], pvA)
                            nc.vector.tensor_copy(un[:, tq * 2 + 1, :], pvB)

                    # finalize: one batched reciprocal for all 8 slots
                    with nc.named_scope(f"fin{pair}"):
                        rec = fpool.tile([1, 8, 512], F32, tag="rec")
                        nc.vector.reciprocal(rec, un[64:65, :, :])
                        for tq in range(4):
                            for h in range(2):
                                slot = tq * 2 + h
                                prt = h * 64
                                bc = auxp.tile([64, 512], F32, tag="aux")
                                nc.tensor.matmul(
                                    bc, ones64_32[0:1, :], rec[:, slot, :],
                                    start=True, stop=True)
                                bcs = opool.tile([64, 512], F32, tag="bcs")
                                nc.vector.tensor_copy(bcs, bc)
                                nc.vector.tensor_mul(
                                    attn[prt:prt + 64, pair,
                                         bass.ds(tq * 512, 512)],
                                    un[0:64, slot, :],
                                    bcs)

                    # out_proj partial for this pair (overlaps next pair's B)
                    with nc.named_scope(f"oproj{pair}"):
                        for fc in range(NEC):
                            ost = opool.tile([128, S], F32, tag="ost")
                            for sb in range(NSB):
                                ps = auxp.tile([128, 512], F32, tag="aux")
                                nc.tensor.matmul(
                                    ps,
                                    wout[:, pair, bass.ts(fc, 128)],
                                    attn[:, pair, bass.ts(sb, 512)],
                                    start=True, stop=True)
                                if pair == 0:
                                    nc.vector.tensor_copy(
                                        ost[:, bass.ts(sb, 512)], ps)
                                else:
                                    nc.scalar.copy(
                                        ost[:, bass.ts(sb, 512)], ps)
                            nc.sync.dma_start(
                                out=outT[pair, bass.ts(fc, 128), :], in_=ost)
    _split_excess_waits(nc)
    return nc


_NC_CACHE = None


def _get_nc():
    global _NC_CACHE
    if _NC_CACHE is None:
        _NC_CACHE = _build_nc()
    return _NC_CACHE


def kernel(x, in_proj_weight, in_proj_bias, out_proj_weight, out_proj_bias,
           _run_kwargs=None, _capture=None):
    x = np.asarray(x, dtype=np.float32)
    in_proj_weight = np.asarray(in_proj_weight, dtype=np.float32)
    in_proj_bias = np.asarray(in_proj_bias, dtype=np.float32)
    out_proj_weight = np.asarray(out_proj_weight, dtype=np.float32)
    out_proj_bias = np.asarray(out_proj_bias, dtype=np.float32)

    nc = _get_nc()
    xTb = [np.ascontiguousarray(x[:, b, :].T) for b in range(B)]

    in_maps = []
    for c in range(N_CORES):
        b = c // 4
        h0 = (c % 4) * 4
        rows = slice(h0 * D, h0 * D + 4 * D)
        wq = in_proj_weight[0:E][rows]          # [256, 1024]
        wk = in_proj_weight[E:2 * E][rows]
        wv_ = in_proj_weight[2 * E:3 * E][rows]
        wqkT = np.ascontiguousarray(np.concatenate([wq, wk], axis=0).T)
        wvT = np.ascontiguousarray(wv_.T)
        woutT = np.ascontiguousarray(out_proj_weight[:, rows].T)
        bqk = np.concatenate(
            [in_proj_bias[0:E][rows], in_proj_bias[E:2 * E][rows]])
        bias_qk = np.ascontiguousarray(bqk.reshape(4, 128).T)
        bias_v = in_proj_bias[2 * E:3 * E][rows].reshape(1, 256)
        in_maps.append({
            "xT": xTb[b],
            "wqkT": wqkT,
            "wvT": wvT,
            "woutT": woutT,
            "bias_qk": bias_qk,
            "bias_v": np.ascontiguousarray(bias_v),
        })

    res = run_bass_kernel_spmd(nc, in_maps, core_ids=list(range(N_CORES)),
                               **(_run_kwargs or {}))
    if _capture is not None:
        _capture["res"] = res

    out = np.zeros((S, B, E), dtype=np.float32)
    for c in range(N_CORES):
        b = c // 4
        o = res.results[c]["outT"]
        out[:, b, :] += o[0].T
        out[:, b, :] += o[1].T
    out += out_proj_bias
    return out


# revision 6
# speedup vs baseline: 1.4826x; 1.1506x over previous
"""MultiHeadAttention forward on 8 TRN2 NeuronCores (batch*head parallel).

Problem: S=2048, B=2, E=1024, H=16 heads, D=64. Each core handles one batch
(b = core//4) and 4 consecutive heads ((core%4)*4 ...), as 2 head-pairs.

Per-core program (all matmuls float32r: 1 cycle/row at N>=256, ~1e-3 rel err):
  Phase A: QKV projection. Host pre-transposes x and weight slices so the
    contraction dim (E) lands on SBUF partitions. Q^T/K^T in feature-major
    [f, s] layout; V in natural [s, d] layout with an appended ones column
    (the softmax denominator drops out of the P@V matmul).
  Phase B: per head-pair, per t-quarter (512): row-packed K=64 score matmuls
    (heads at array rows 0-63/64-127 run concurrently), one ACT exp over the
    combined [128, 1024] PSUM strip (scale=1/8 folded in), P@V accumulation
    with [V|1] stationary, then a PSUM->SBUF staging copy (frees the PSUM
    bank immediately; normalization is deferred).
  Finalize (per pair, off critical path): ONE batched DVE reciprocal over all
    8 denominator rows, PE ones-broadcast, DVE multiply.
  Phase C: out_proj partials per pair (pair 0 overlaps phase B of pair 1);
    host sums the 2x4 partials per batch and adds out_proj_bias.
"""
import sys

if "/opt/trn_rl_repo" not in sys.path:
    sys.path.insert(0, "/opt/trn_rl_repo")

import numpy as np

import concourse.bass as bass
import concourse.tile as tile
from concourse import mybir
from concourse.bass_utils import run_bass_kernel_spmd

S = 2048
B = 2
E = 1024
H = 16
D = 64
N_CORES = 8
F32 = mybir.dt.float32
F32R = mybir.dt.float32r
EXP = mybir.ActivationFunctionType.Exp
SCALING = float(D) ** -0.5

NSCH = S // 128   # 16 s-chunks
NSB = S // 512    # 4 s-blocks
NEC = E // 128    # 8 e-chunks


def _split_excess_waits(nc, limit=1):
    """This walrus build accepts at most 2 sync-wait commands per instruction;
    hoist excess waits onto preceding same-engine NOPs (queue order preserves
    semantics)."""
    ctr = 0
    for f in nc.m.functions:
        for blk in f.blocks:
            insts = blk.instructions
            if not any(
                i.sync_info and i.sync_info.on_wait and len(i.sync_info.on_wait) > limit
                for i in insts
            ):
                continue
            out = []
            for inst in insts:
                si = inst.sync_info
                if si and si.on_wait and len(si.on_wait) > limit:
                    waits = list(si.on_wait)
                    excess, keep = waits[:-limit], waits[-limit:]
                    for i in range(0, len(excess), limit):
                        ctr += 1
                        nop = mybir.InstNoOp(name=f"waitsplit-nop-{ctr}")
                        nop.engine = inst.engine
                        nop.sync_info = mybir.SyncInfo(
                            on_wait=excess[i : i + limit], on_update=[]
                        )
                        nc.register_instruction(nop, overwrite=True)
                        out.append(nop)
                    si.on_wait = keep
                out.append(inst)
            blk.instructions.clear()
            blk.instructions.extend(out)
    return nc


def _build_nc():
    nc = bass.Bass()
    # f32r-typed DRAM inputs: numpy float32 bytes, HWDGE plain DMA, PE rounds.
    xT = nc.dram_tensor("xT", [E, S], F32R, kind="ExternalInput")
    wqkT = nc.dram_tensor("wqkT", [E, 512], F32R, kind="ExternalInput")
    wvT = nc.dram_tensor("wvT", [E, 256], F32R, kind="ExternalInput")
    woutT = nc.dram_tensor("woutT", [256, E], F32R, kind="ExternalInput")
    bias_qk = nc.dram_tensor("bias_qk", [128, 4], F32, kind="ExternalInput")
    bias_v = nc.dram_tensor("bias_v", [1, 256], F32, kind="ExternalInput")
    outT = nc.dram_tensor("outT", [2, E, S], F32, kind="ExternalOutput")

    with tile.TileContext(nc) as tc:
        with tc.tile_pool(name="wpool", bufs=1) as wpool, \
             tc.tile_pool(name="qkpool", bufs=1) as qkpool, \
             tc.tile_pool(name="vapool", bufs=1) as vapool, \
             tc.tile_pool(name="attnpool", bufs=1) as attnpool, \
             tc.tile_pool(name="auxpsum", bufs=2, space="PSUM") as auxp:
            # ---- constants / weights ----
            wqk = wpool.tile([128, NEC, 512], F32R)
            nc.sync.dma_start(
                out=wqk, in_=wqkT.rearrange("(c p) f -> p c f", p=128))
            wv = wpool.tile([128, NEC, 256], F32R)
            nc.sync.dma_start(
                out=wv, in_=wvT.rearrange("(c p) f -> p c f", p=128))
            wout = wpool.tile([128, 2, E], F32R)
            nc.sync.dma_start(
                out=wout, in_=woutT.rearrange("(c p) f -> p c f", p=128))
            bqk = wpool.tile([128, 4], F32)
            nc.sync.dma_start(out=bqk, in_=bias_qk[:, :])
            bv = wpool.tile([128, 256], F32)
            nc.sync.dma_start(out=bv, in_=bias_v[:, :].to_broadcast([128, 256]))
            ones64_32 = wpool.tile([128, 64], F32)
            nc.vector.memset(ones64_32, 1.0)
            onesbc = wpool.tile([1, 64], F32R)
            nc.vector.tensor_copy(onesbc, ones64_32[0:1, :])

            # persistent activations
            qk = qkpool.tile([128, 4, S], F32R)        # Q^T (chunks 0-1), K^T (2-3)
            va = vapool.tile([128, NSCH, 4, 65], F32R)  # V natural + ones col
            attn = attnpool.tile([128, 2, S], F32R)    # attn^T normalized

            nc.vector.tensor_copy(
                va[:, :, :, 64:65],
                ones64_32.rearrange("p (c h) -> p c h", h=4).unsqueeze(3))

            # ---- phase A: QKV projection ----
            with nc.named_scope("phaseA"), \
                 tc.tile_pool(name="xpool", bufs=1) as xpool, \
                 tc.tile_pool(name="apsum", bufs=2, space="PSUM") as apsum:
                xt = xpool.tile([128, NEC, S], F32R)
                for ec in range(NEC):
                    nc.sync.dma_start(
                        out=xt[:, ec, :], in_=xT[bass.ts(ec, 128), :])

                for fc in range(4):
                    for sb in range(NSB):
                        ps = apsum.tile([128, 512], F32, tag="qkps")
                        for ec in range(NEC):
                            nc.tensor.matmul(
                                ps,
                                wqk[:, ec, bass.ts(fc, 128)],
                                xt[:, ec, bass.ts(sb, 512)],
                                start=(ec == 0), stop=(ec == NEC - 1))
                        nc.vector.tensor_scalar(
                            out=qk[:, fc, bass.ts(sb, 512)], in0=ps,
                            scalar1=bqk[:, fc:fc + 1], scalar2=None,
                            op0=mybir.AluOpType.add)

                for i in range(NSCH):
                    ps = apsum.tile([128, 256], F32, tag="vps")
                    for ec in range(NEC):
                        nc.tensor.matmul(
                            ps,
                            xt[:, ec, bass.ts(i, 128)],
                            wv[:, ec, :],
                            start=(ec == 0), stop=(ec == NEC - 1))
                    nc.vector.tensor_tensor(
                        out=va[:, i, :, 0:64],
                        in0=ps.rearrange("p (h d) -> p h d", h=4),
                        in1=bv.rearrange("p (h d) -> p h d", h=4),
                        op=mybir.AluOpType.add)

            # ---- phase B + finalize + out_proj, pair by pair ----
            with tc.tile_pool(name="ppool", bufs=3) as ppool, \
                 tc.tile_pool(name="unpool", bufs=2) as unpool, \
                 tc.tile_pool(name="fpool", bufs=1) as fpool, \
                 tc.tile_pool(name="opool", bufs=2) as opool, \
                 tc.tile_pool(name="scpsum", bufs=2, space="PSUM") as scp, \
                 tc.tile_pool(name="pvpsum", bufs=1, space="PSUM") as pvp:
                for pair in range(2):
                    hA, hB = 2 * pair, 2 * pair + 1
                    qc = pair       # Q chunk of this pair
                    kc = 2 + pair   # K chunk
                    # unnormalized P@V results + sums: 8 slots of [65, 512]
                    # (slot = tq*2 + head), row 64 = denominators
                    un = unpool.tile([65, 8, 512], F32, tag="un")
                    with nc.named_scope(f"scores{pair}"):
                        for tq in range(4):
                            toff = tq * 512
                            pvA = pvp.tile([65, 512], F32, tag="pvA")
                            pvB = pvp.tile([65, 512], F32, tag="pvB")
                            for i in range(NSCH):
                                sc = scp.tile([128, 1024], F32, tag="sc")
                                nc.tensor.matmul(
                                    sc[:, 0:512],
                                    qk[0:64, kc, bass.ts(i, 128)],
                                    qk[0:64, qc, bass.ds(toff, 512)],
                                    start=True, stop=True)
                                nc.tensor.matmul(
                                    sc[:, 512:1024],
                                    qk[64:128, kc, bass.ts(i, 128)],
                                    qk[64:128, qc, bass.ds(toff, 512)],
                                    start=True, stop=True)
                                p = ppool.tile([128, 1024], F32R, tag="p")
                                nc.scalar.activation(p, sc, EXP, scale=SCALING)
                                nc.tensor.matmul(
                                    pvA, va[:, i, hA, :], p[:, 0:512],
                                    start=(i == 0), stop=(i == NSCH - 1))
                                nc.tensor.matmul(
                                    pvB, va[:, i, hB, :], p[:, 512:1024],
                                    start=(i == 0), stop=(i == NSCH - 1))
                            nc.vector.tensor_copy(un[:, tq * 2, :], pvA)
                            nc.vector.tensor_copy(un[:, tq * 2 + 1, :], pvB)

                    # finalize: DVE reciprocal costs ~6.3ns per free-dim
                    # element (lane-parallel over partitions only), so shuffle
                    # the 4096 sums across 128 partitions via SBUF->SBUF DMA,
                    # take the reciprocal there, and shuffle back.
                    with nc.named_scope(f"fin{pair}"):
                        recin = fpool.tile([128, 32], F32, tag="recin")
                        nc.sync.dma_start(out=recin, in_=un[64:65, :, :])
                        recw = fpool.tile([128, 32], F32, tag="recw")
                        nc.vector.reciprocal(recw, recin)
                        recwr = fpool.tile([128, 32], F32R, tag="recwr")
                        nc.vector.tensor_copy(recwr, recw)
                        rec = fpool.tile([1, 8, 512], F32R, tag="rec")
                        nc.sync.dma_start(out=rec, in_=recwr)
                        for tq in range(4):
                            for h in range(2):
                                slot = tq * 2 + h
                                prt = h * 64
                                bc = auxp.tile([64, 512], F32, tag="aux")
                                nc.tensor.matmul(
                                    bc, onesbc, rec[:, slot, :],
                                    start=True, stop=True)
                                bcs = opool.tile([64, 512], F32, tag="bcs")
                                nc.vector.tensor_copy(bcs, bc)
                                nc.vector.tensor_mul(
                                    attn[prt:prt + 64, pair,
                                         bass.ds(tq * 512, 512)],
                                    un[0:64, slot, :],
                                    bcs)

                    # out_proj partial for this pair (overlaps next pair's B)
                    with nc.named_scope(f"oproj{pair}"):
                        for fc in range(NEC):
                            ost = opool.tile([128, S], F32, tag="ost")
                            for sb in range(NSB):
                                ps = auxp.tile([128, 512], F32, tag="aux")
                                nc.tensor.matmul(
                                    ps,
                                    wout[:, pair, bass.ts(fc, 128)],
                                    attn[:, pair, bass.ts(sb, 512)],
                                    start=True, stop=True)
                                if pair == 0:
                                    nc.vector.tensor_copy(
                                        ost[:, bass.ts(sb, 512)], ps)
                                else:
                                    nc.scalar.copy(
                                        ost[:, bass.ts(sb, 512)], ps)
                            nc.sync.dma_start(
                                out=outT[pair, bass.ts(fc, 128), :], in_=ost)
    _split_excess_waits(nc)
    return nc


_NC_CACHE = None


def _get_nc():
    global _NC_CACHE
    if _NC_CACHE is None:
        _NC_CACHE = _build_nc()
    return _NC_CACHE


def kernel(x, in_proj_weight, in_proj_bias, out_proj_weight, out_proj_bias,
           _run_kwargs=None, _capture=None):
    x = np.asarray(x, dtype=np.float32)
    in_proj_weight = np.asarray(in_proj_weight, dtype=np.float32)
    in_proj_bias = np.asarray(in_proj_bias, dtype=np.float32)
    out_proj_weight = np.asarray(out_proj_weight, dtype=np.float32)
    out_proj_bias = np.asarray(out_proj_bias, dtype=np.float32)

    nc = _get_nc()
    xTb = [np.ascontiguousarray(x[:, b, :].T) for b in range(B)]

    in_maps = []
    for c in range(N_CORES):
        b = c // 4
        h0 = (c % 4) * 4
        rows = slice(h0 * D, h0 * D + 4 * D)
        wq = in_proj_weight[0:E][rows]          # [256, 1024]
        wk = in_proj_weight[E:2 * E][rows]
        wv_ = in_proj_weight[2 * E:3 * E][rows]
        wqkT = np.ascontiguousarray(np.concatenate([wq, wk], axis=0).T)
        wvT = np.ascontiguousarray(wv_.T)
        woutT = np.ascontiguousarray(out_proj_weight[:, rows].T)
        bqk = np.concatenate(
            [in_proj_bias[0:E][rows], in_proj_bias[E:2 * E][rows]])
        bias_qk = np.ascontiguousarray(bqk.reshape(4, 128).T)
        bias_v = in_proj_bias[2 * E:3 * E][rows].reshape(1, 256)
        in_maps.append({
            "xT": xTb[b],
            "wqkT": wqkT,
            "wvT": wvT,
            "woutT": woutT,
            "bias_qk": bias_qk,
            "bias_v": np.ascontiguousarray(bias_v),
        })

    res = run_bass_kernel_spmd(nc, in_maps, core_ids=list(range(N_CORES)),
                               **(_run_kwargs or {}))
    if _capture is not None:
        _capture["res"] = res

    out = np.zeros((S, B, E), dtype=np.float32)
    for c in range(N_CORES):
        b = c // 4
        o = res.results[c]["outT"]
        out[:, b, :] += o[0].T
        out[:, b, :] += o[1].T
    out += out_proj_bias
    return out


# revision 8
# speedup vs baseline: 1.6689x; 1.1256x over previous
"""MultiHeadAttention forward on 8 TRN2 NeuronCores (batch*head parallel).

Problem: S=2048, B=2, E=1024, H=16 heads, D=64. Each core handles one batch
(b = core//4) and 4 consecutive heads ((core%4)*4 ...), as 2 head-pairs.

Per-core program (all matmuls float32r: 1 cycle/row at N>=256, ~1e-3 rel err):
  Phase A: QKV projection. Host pre-transposes x and weight slices so the
    contraction dim (E) lands on SBUF partitions. Q^T/K^T in feature-major
    [f, s] layout; V in natural [s, d] layout with an appended ones column
    (the softmax denominator drops out of the P@V matmul).
  Phase B: per head-pair, per t-quarter (512): row-packed K=64 score matmuls
    (heads at array rows 0-63/64-127 run concurrently), one ACT exp over the
    combined [128, 1024] PSUM strip (scale=1/8 folded in), P@V accumulation
    with [V|1] stationary, then a PSUM->SBUF staging copy (frees the PSUM
    bank immediately; normalization is deferred).
  Finalize (per pair, off critical path): ONE batched DVE reciprocal over all
    8 denominator rows, PE ones-broadcast, DVE multiply.
  Phase C: out_proj partials per pair (pair 0 overlaps phase B of pair 1);
    host sums the 2x4 partials per batch and adds out_proj_bias.
"""
import sys

if "/opt/trn_rl_repo" not in sys.path:
    sys.path.insert(0, "/opt/trn_rl_repo")

import numpy as np

import concourse.bass as bass
import concourse.tile as tile
from concourse import mybir
from concourse.bass_utils import run_bass_kernel_spmd

S = 2048
B = 2
E = 1024
H = 16
D = 64
N_CORES = 8
F32 = mybir.dt.float32
F32R = mybir.dt.float32r
EXP = mybir.ActivationFunctionType.Exp
SCALING = float(D) ** -0.5

NSCH = S // 128   # 16 s-chunks
NSB = S // 512    # 4 s-blocks
NEC = E // 128    # 8 e-chunks


def _split_excess_waits(nc, limit=1):
    """This walrus build accepts at most 2 sync-wait commands per instruction;
    hoist excess waits onto preceding same-engine NOPs (queue order preserves
    semantics)."""
    ctr = 0
    for f in nc.m.functions:
        for blk in f.blocks:
            insts = blk.instructions
            if not any(
                i.sync_info and i.sync_info.on_wait and len(i.sync_info.on_wait) > limit
                for i in insts
            ):
                continue
            out = []
            for inst in insts:
                si = inst.sync_info
                if si and si.on_wait and len(si.on_wait) > limit:
                    waits = list(si.on_wait)
                    excess, keep = waits[:-limit], waits[-limit:]
                    for i in range(0, len(excess), limit):
                        ctr += 1
                        nop = mybir.InstNoOp(name=f"waitsplit-nop-{ctr}")
                        nop.engine = inst.engine
                        nop.sync_info = mybir.SyncInfo(
                            on_wait=excess[i : i + limit], on_update=[]
                        )
                        nc.register_instruction(nop, overwrite=True)
                        out.append(nop)
                    si.on_wait = keep
                out.append(inst)
            blk.instructions.clear()
            blk.instructions.extend(out)
    return nc


def _build_nc():
    nc = bass.Bass()
    # f32r-typed DRAM inputs: numpy float32 bytes, HWDGE plain DMA, PE rounds.
    xT = nc.dram_tensor("xT", [E, S], F32R, kind="ExternalInput")
    wqkT = nc.dram_tensor("wqkT", [E, 512], F32R, kind="ExternalInput")
    wvT = nc.dram_tensor("wvT", [E, 256], F32R, kind="ExternalInput")
    woutT = nc.dram_tensor("woutT", [256, E], F32R, kind="ExternalInput")
    bias_qk = nc.dram_tensor("bias_qk", [128, 4], F32, kind="ExternalInput")
    bias_v = nc.dram_tensor("bias_v", [1, 256], F32, kind="ExternalInput")
    outT = nc.dram_tensor("outT", [2, E, S], F32, kind="ExternalOutput")

    with tile.TileContext(nc) as tc:
        with tc.tile_pool(name="wpool", bufs=1) as wpool, \
             tc.tile_pool(name="qkpool", bufs=1) as qkpool, \
             tc.tile_pool(name="vapool", bufs=1) as vapool, \
             tc.tile_pool(name="attnpool", bufs=1) as attnpool, \
             tc.tile_pool(name="auxpsum", bufs=2, space="PSUM") as auxp:
            # ---- constants / weights (wout is loaded last, it is needed
            # last; x chunks + QK weights load first so the PE can start) ----
            wqk = wpool.tile([128, NEC, 512], F32R)
            nc.sync.dma_start(
                out=wqk, in_=wqkT.rearrange("(c p) f -> p c f", p=128))
            wv = wpool.tile([128, NEC, 256], F32R)
            wout = wpool.tile([128, 2, E], F32R)
            bqk = wpool.tile([128, 4], F32)
            nc.sync.dma_start(out=bqk, in_=bias_qk[:, :])
            bv = wpool.tile([128, 256], F32)
            nc.sync.dma_start(out=bv, in_=bias_v[:, :].to_broadcast([128, 256]))
            ones64_32 = wpool.tile([128, 64], F32)
            nc.vector.memset(ones64_32, 1.0)
            onesbc = wpool.tile([1, 64], F32R)
            nc.vector.tensor_copy(onesbc, ones64_32[0:1, :])

            # persistent activations
            qk = qkpool.tile([128, 4, S], F32R)        # Q^T (chunks 0-1), K^T (2-3)
            va = vapool.tile([128, NSCH, 4, 65], F32R)  # V natural + ones col
            attn = attnpool.tile([128, 2, S], F32R)    # attn^T normalized

            nc.vector.tensor_copy(
                va[:, :, :, 64:65],
                ones64_32.rearrange("p (c h) -> p c h", h=4).unsqueeze(3))

            # ---- phase A: QKV projection ----
            with nc.named_scope("phaseA"), \
                 tc.tile_pool(name="xpool", bufs=1) as xpool, \
                 tc.tile_pool(name="apsum", bufs=2, space="PSUM") as apsum:
                xt = xpool.tile([128, NEC, S], F32R)
                for ec in range(NEC):
                    nc.sync.dma_start(
                        out=xt[:, ec, :], in_=xT[bass.ts(ec, 128), :])
                nc.sync.dma_start(
                    out=wv, in_=wvT.rearrange("(c p) f -> p c f", p=128))
                nc.sync.dma_start(
                    out=wout, in_=woutT.rearrange("(c p) f -> p c f", p=128))

                for fc in range(4):
                    for sb in range(NSB):
                        ps = apsum.tile([128, 512], F32, tag="qkps")
                        for ec in range(NEC):
                            nc.tensor.matmul(
                                ps,
                                wqk[:, ec, bass.ts(fc, 128)],
                                xt[:, ec, bass.ts(sb, 512)],
                                start=(ec == 0), stop=(ec == NEC - 1))
                        nc.vector.tensor_scalar(
                            out=qk[:, fc, bass.ts(sb, 512)], in0=ps,
                            scalar1=bqk[:, fc:fc + 1], scalar2=None,
                            op0=mybir.AluOpType.add)

                for i in range(NSCH):
                    ps = apsum.tile([128, 256], F32, tag="vps")
                    for ec in range(NEC):
                        nc.tensor.matmul(
                            ps,
                            xt[:, ec, bass.ts(i, 128)],
                            wv[:, ec, :],
                            start=(ec == 0), stop=(ec == NEC - 1))
                    nc.vector.tensor_tensor(
                        out=va[:, i, :, 0:64],
                        in0=ps.rearrange("p (h d) -> p h d", h=4),
                        in1=bv.rearrange("p (h d) -> p h d", h=4),
                        op=mybir.AluOpType.add)

            # ---- phase B + finalize + out_proj, pair by pair ----
            with tc.tile_pool(name="ppool", bufs=3) as ppool, \
                 tc.tile_pool(name="unpool", bufs=3) as unpool, \
                 tc.tile_pool(name="fpool", bufs=2) as fpool, \
                 tc.tile_pool(name="opool", bufs=4) as opool, \
                 tc.tile_pool(name="scpsum", bufs=2, space="PSUM") as scp, \
                 tc.tile_pool(name="pvpsum", bufs=1, space="PSUM") as pvp:
                for pair in range(2):
                    hA, hB = 2 * pair, 2 * pair + 1
                    qc = pair       # Q chunk of this pair
                    kc = 2 + pair   # K chunk
                    for tq in range(4):
                        toff = tq * 512
                        pvA = pvp.tile([65, 512], F32, tag="pvA")
                        pvB = pvp.tile([65, 512], F32, tag="pvB")
                        with nc.named_scope(f"scores{pair}_{tq}"):
                            for i in range(NSCH):
                                sc = scp.tile([128, 1024], F32, tag="sc")
                                nc.tensor.matmul(
                                    sc[:, 0:512],
                                    qk[0:64, kc, bass.ts(i, 128)],
                                    qk[0:64, qc, bass.ds(toff, 512)],
                                    start=True, stop=True)
                                nc.tensor.matmul(
                                    sc[:, 512:1024],
                                    qk[64:128, kc, bass.ts(i, 128)],
                                    qk[64:128, qc, bass.ds(toff, 512)],
                                    start=True, stop=True)
                                p = ppool.tile([128, 1024], F32R, tag="p")
                                nc.scalar.activation(p, sc, EXP, scale=SCALING)
                                nc.tensor.matmul(
                                    pvA, va[:, i, hA, :], p[:, 0:512],
                                    start=(i == 0), stop=(i == NSCH - 1))
                                nc.tensor.matmul(
                                    pvB, va[:, i, hB, :], p[:, 512:1024],
                                    start=(i == 0), stop=(i == NSCH - 1))
                        # finalize this t-quarter: stage unnormalized P@V +
                        # sums to SBUF (frees PSUM), reciprocal via partition
                        # shuffle (DVE recip costs ~6.3ns/free-elem), ones-
                        # broadcast matmul, normalize.
                        with nc.named_scope(f"fin{pair}_{tq}"):
                            un = unpool.tile([65, 2, 512], F32, tag="un")
                            nc.vector.tensor_copy(un[:, 0, :], pvA)
                            nc.vector.tensor_copy(un[:, 1, :], pvB)
                            recin = fpool.tile([128, 8], F32, tag="recin")
                            nc.sync.dma_start(out=recin, in_=un[64:65, :, :])
                            recw = fpool.tile([128, 8], F32, tag="recw")
                            nc.vector.reciprocal(recw, recin)
                            recwr = fpool.tile([128, 8], F32R, tag="recwr")
                            nc.vector.tensor_copy(recwr, recw)
                            rec = fpool.tile([1, 2, 512], F32R, tag="rec")
                            nc.sync.dma_start(out=rec, in_=recwr)
                            for h in range(2):
                                prt = h * 64
                                bc = auxp.tile([64, 512], F32, tag="aux")
                                nc.tensor.matmul(
                                    bc, onesbc, rec[:, h, :],
                                    start=True, stop=True)
                                bcs = opool.tile([64, 512], F32, tag="bcs")
                                nc.vector.tensor_copy(bcs, bc)
                                nc.vector.tensor_mul(
                                    attn[prt:prt + 64, pair,
                                         bass.ds(toff, 512)],
                                    un[0:64, h, :],
                                    bcs)
                        # out_proj column sb == tq for this pair
                        with nc.named_scope(f"oproj{pair}_{tq}"):
                            for fc in range(NEC):
                                ps = auxp.tile([128, 512], F32, tag="aux")
                                nc.tensor.matmul(
                                    ps,
                                    wout[:, pair, bass.ts(fc, 128)],
                                    attn[:, pair, bass.ds(toff, 512)],
                                    start=True, stop=True)
                                ocp = opool.tile([128, 512], F32, tag="ocp")
                                nc.vector.tensor_copy(ocp, ps)
                                nc.sync.dma_start(
                                    out=outT[pair, bass.ts(fc, 128),
                                             bass.ds(toff, 512)],
                                    in_=ocp)
    _split_excess_waits(nc)
    return nc


_NC_CACHE = None


def _get_nc():
    global _NC_CACHE
    if _NC_CACHE is None:
        _NC_CACHE = _build_nc()
    return _NC_CACHE


def kernel(x, in_proj_weight, in_proj_bias, out_proj_weight, out_proj_bias,
           _run_kwargs=None, _capture=None):
    x = np.asarray(x, dtype=np.float32)
    in_proj_weight = np.asarray(in_proj_weight, dtype=np.float32)
    in_proj_bias = np.asarray(in_proj_bias, dtype=np.float32)
    out_proj_weight = np.asarray(out_proj_weight, dtype=np.float32)
    out_proj_bias = np.asarray(out_proj_bias, dtype=np.float32)

    nc = _get_nc()
    xTb = [np.ascontiguousarray(x[:, b, :].T) for b in range(B)]

    in_maps = []
    for c in range(N_CORES):
        b = c // 4
        h0 = (c % 4) * 4
        rows = slice(h0 * D, h0 * D + 4 * D)
        wq = in_proj_weight[0:E][rows]          # [256, 1024]
        wk = in_proj_weight[E:2 * E][rows]
        wv_ = in_proj_weight[2 * E:3 * E][rows]
        wqkT = np.ascontiguousarray(np.concatenate([wq, wk], axis=0).T)
        wvT = np.ascontiguousarray(wv_.T)
        woutT = np.ascontiguousarray(out_proj_weight[:, rows].T)
        bqk = np.concatenate(
            [in_proj_bias[0:E][rows], in_proj_bias[E:2 * E][rows]])
        bias_qk = np.ascontiguousarray(bqk.reshape(4, 128).T)
        bias_v = in_proj_bias[2 * E:3 * E][rows].reshape(1, 256)
        in_maps.append({
            "xT": xTb[b],
            "wqkT": wqkT,
            "wvT": wvT,
            "woutT": woutT,
            "bias_qk": bias_qk,
            "bias_v": np.ascontiguousarray(bias_v),
        })

    res = run_bass_kernel_spmd(nc, in_maps, core_ids=list(range(N_CORES)),
                               **(_run_kwargs or {}))
    if _capture is not None:
        _capture["res"] = res

    out = np.zeros((S, B, E), dtype=np.float32)
    for c in range(N_CORES):
        b = c // 4
        o = res.results[c]["outT"]
        out[:, b, :] += o[0].T
        out[:, b, :] += o[1].T
    out += out_proj_bias
    return out
